# revision 13
# baseline (speedup 1.0000x reference)
"""Trainium2 Bass kernel for nn_AttentionBlock (B=16, C=512, H=W=64, 8 heads).

Channel-attention block: GroupNorm(8 groups) -> 1x1 qkv -> scores over
channel dims (contract spatial N=4096) -> softmax -> att @ v -> 1x1 out
projection -> residual.

Sharding: data-parallel over batch, 2 per core, no collectives.

Key structure (vs a direct port of the reference):
  * scores are computed via the Gram matrix G = h @ h^T:
        S = Wq G Wk^T + qs x bk + bq x (ks + N*bk),  qs/ks = Wq/Wk @ rowsum(h)
    which costs C*C*N MACs (half of the q,k projections) and removes the
    [N, 2C] qk psum evacuation entirely.  G is built from PE transposes of
    h (spatial-on-partition tiles) accumulated over 32 spatial chunks.
  * v and out projections run in fp8 (e4m3) DoubleRow perf mode: weights
    are scaled by 32 host-side (avoids subnormals), inputs h/hv are stored
    fp8 in a paired-chunk layout [128, 2, N], and each matmul contracts
    256 channels at 2x throughput.  The 1/32 unscale folds into the psum
    evacuations.  (q,k stay bf16: softmax amplifies fp8 noise there.)
  * att is kept block-diagonal per channel-chunk ([128,128] tiles with two
    64x64 head blocks on the diagonal), so att@v and att@b_v are single
    full-width 128-contraction matmuls per (chunk, t-block).
  * x is loaded in bf16 (halves the startup DMA); the residual re-loads
    x in fp32 per 512-col block.
  * engine split: PE matmuls; ACT normalize+v/out/G/T evacs+exp; DVE
    bn_stats+hT/hv evacs+softmax small ops; Pool h8 normalize, residual
    adds and output stores.
"""

import numpy as np
import ml_dtypes

import concourse.bacc as bacc
import concourse.tile as tile
from concourse import mybir
from concourse.bass_utils import run_bass_kernel_spmd
from concourse.masks import make_identity

BF = mybir.dt.bfloat16
F8 = mybir.dt.float8e4
F32 = mybir.dt.float32
AX = mybir.AxisListType
OP = mybir.AluOpType
AF = mybir.ActivationFunctionType
DR = mybir.MatmulPerfMode.DoubleRow

C = 512
NH = 8
D = 64
G = 8
CK = C // 128   # 4 channel chunks
NP = 2          # chunk pairs for fp8 DoubleRow
EPS = 1e-5
N_CORES = 8
WSC = 32.0      # fp8 weight scale
IWSC = float(1.0 / WSC)


def build_program(B=2, N=4096, debug=False):
    SP = N // 128   # 32 spatial chunks
    NT = N // 512   # 8 t-blocks
    SUB = N // 512
    scale = float(1.0 / np.sqrt(D))

    nc = bacc.Bacc("TRN2", target_bir_lowering=False, debug=debug,
                   num_devices=N_CORES)

    xbf_d = nc.dram_tensor("xbf", [B, C, N], BF, kind="ExternalInput")
    xf_d = nc.dram_tensor("xf", [B, C, N], F32, kind="ExternalInput")
    wqkT_d = nc.dram_tensor("wqkT", [C, 2 * C], BF, kind="ExternalInput")
    wv8_d = nc.dram_tensor("wv8", [NP, 128, 2, C], F8, kind="ExternalInput")
    wo8_d = nc.dram_tensor("wo8", [NP, 128, 2, C], F8, kind="ExternalInput")
    bqk_d = nc.dram_tensor("bqkr", [1, 2 * C], BF, kind="ExternalInput")
    bv_d = nc.dram_tensor("bv", [C, 1], BF, kind="ExternalInput")
    bo_d = nc.dram_tensor("bo", [C, 1], F32, kind="ExternalInput")
    gam_d = nc.dram_tensor("gamma", [C, 1], F32, kind="ExternalInput")
    bet_d = nc.dram_tensor("beta", [C, 1], F32, kind="ExternalInput")
    indf_d = nc.dram_tensor("indf", [C, G], F32, kind="ExternalInput")
    indb_d = nc.dram_tensor("indb", [G, C], F32, kind="ExternalInput")
    out_d = nc.dram_tensor("out", [B, C, N], F32, kind="ExternalOutput")

    with tile.TileContext(nc) as tc:
        import contextlib
        ctx = contextlib.ExitStack()
        with ctx:
            persist = ctx.enter_context(tc.tile_pool(name="persist", bufs=1))
            xpool = ctx.enter_context(tc.tile_pool(name="xpool", bufs=1))
            hpool = ctx.enter_context(tc.tile_pool(name="hpool", bufs=1))
            vpool = ctx.enter_context(tc.tile_pool(name="vpool", bufs=1))
            gpool = ctx.enter_context(tc.tile_pool(name="gpool", bufs=1))
            mid = ctx.enter_context(tc.tile_pool(name="mid", bufs=3))
            small = ctx.enter_context(tc.tile_pool(name="small", bufs=1))
            # PSUM: G 4 banks (also att@v via tag rotation) + tr 2 +
            # wk 1 (vproj/stats/T) + av 1 (qks/scores/cv/outproj) = 8 banks
            ps_g = ctx.enter_context(
                tc.tile_pool(name="ps_g", bufs=1, space="PSUM"))
            ps_tr = ctx.enter_context(
                tc.tile_pool(name="ps_tr", bufs=2, space="PSUM"))
            ps_w = ctx.enter_context(
                tc.tile_pool(name="ps_w", bufs=1, space="PSUM"))

            # ---- persistent weights / constants ----
            wqk = []
            for k in range(CK):
                t = persist.tile([128, 2 * C], BF, tag=f"wqk{k}")
                nc.gpsimd.dma_start(out=t, in_=wqkT_d.ap()[k * 128:(k + 1) * 128, :])
                wqk.append(t)
            wv8 = []
            wo8 = []
            for p in range(NP):
                t = persist.tile([128, 2, C], F8, tag=f"wv8{p}")
                nc.gpsimd.dma_start(out=t, in_=wv8_d.ap()[p])
                wv8.append(t)
                t = persist.tile([128, 2, C], F8, tag=f"wo8{p}")
                nc.gpsimd.dma_start(out=t, in_=wo8_d.ap()[p])
                wo8.append(t)
            bqkr = persist.tile([1, 2 * C], BF, tag="bqkr")
            nc.gpsimd.dma_start(out=bqkr, in_=bqk_d.ap())
            bv_sb = []
            bo_sb = []
            gam = []
            bet = []
            indf = []
            for k in range(CK):
                t = persist.tile([128, 1], BF, tag=f"bv{k}")
                nc.gpsimd.dma_start(out=t, in_=bv_d.ap()[k * 128:(k + 1) * 128, :])
                bv_sb.append(t)
                t = persist.tile([128, 1], F32, tag=f"bo{k}")
                nc.gpsimd.dma_start(out=t, in_=bo_d.ap()[k * 128:(k + 1) * 128, :])
                bo_sb.append(t)
                t = persist.tile([128, 1], F32, tag=f"gam{k}")
                nc.gpsimd.dma_start(out=t, in_=gam_d.ap()[k * 128:(k + 1) * 128, :])
                gam.append(t)
                t = persist.tile([128, 1], F32, tag=f"bet{k}")
                nc.gpsimd.dma_start(out=t, in_=bet_d.ap()[k * 128:(k + 1) * 128, :])
                bet.append(t)
                t = persist.tile([128, G], F32, tag=f"indf{k}")
                nc.gpsimd.dma_start(out=t, in_=indf_d.ap()[k * 128:(k + 1) * 128, :])
                indf.append(t)
            indb = persist.tile([G, C], F32, tag="indb")
            nc.gpsimd.dma_start(out=indb, in_=indb_d.ap())
            ident = persist.tile([128, 128], BF, tag="ident")
            make_identity(nc, ident)
            eps_t = persist.tile([128, 1], F32, tag="eps")
            nc.gpsimd.memset(eps_t, EPS)
            # block-diag att tiles: off-diagonal quadrants stay zero forever
            att_bf = []
            for k in range(CK):
                t = persist.tile([128, 128], BF, tag=f"attb{k}")
                nc.gpsimd.memset(t, 0.0)
                att_bf.append(t)

            # ---------------- phase helpers ----------------
            def load_x(b):
                xs = []
                for k in range(CK):
                    t = xpool.tile([128, N], BF, tag=f"x{k}")
                    for q2 in range(0, N, 2048):
                        nc.sync.dma_start(
                            out=t[:, q2:q2 + 2048],
                            in_=xbf_d.ap()[b, k * 128:(k + 1) * 128,
                                           q2:q2 + 2048])
                    xs.append(t)
                return xs

            def emit_bn_stats(xs, k, j):
                # one bn_stats op on a 512-col block; st tile per chunk
                if j == 0:
                    st = small.tile([128, SUB, 6], F32, tag=f"st{k}")
                    emit_bn_stats.st[k] = st
                st = emit_bn_stats.st[k]
                nc.vector.bn_stats(out=st[:, j, :],
                                   in_=xs[k][:, j * 512:(j + 1) * 512])
            emit_bn_stats.st = [None] * CK

            def stats_finish(b):
                """bn_aggr + group reduce -> per-channel sc/nb coeffs."""
                mvs = []
                rhs2s = []
                for k in range(CK):
                    mv = small.tile([128, 2], F32, tag=f"mv{k}")
                    nc.vector.bn_aggr(out=mv, in_=emit_bn_stats.st[k])
                    mvs.append(mv)
                    r2 = small.tile([128, 2], F32, tag=f"r2{k}")
                    nc.gpsimd.tensor_copy(out=r2[:, 0:1], in_=mv[:, 0:1])
                    nc.vector.scalar_tensor_tensor(
                        out=r2[:, 1:2], in0=mv[:, 0:1],
                        scalar=mv[:, 0:1], in1=mv[:, 1:2],
                        op0=OP.mult, op1=OP.add)
                    rhs2s.append(r2)
                pg_t = ps_w.tile([128, 512], F32, tag="wk")
                pg = pg_t[0:G, 0:2]
                for k in range(CK):
                    nc.tensor.matmul(pg, indf[k], rhs2s[k],
                                     start=(k == 0), stop=(k == CK - 1))
                sg = small.tile([G, 2], F32, tag="sg")
                nc.vector.tensor_copy(out=sg, in_=pg)
                t2 = small.tile([G, 1], F32, tag="t2")
                nc.vector.tensor_mul(out=t2, in0=sg[:, 0:1], in1=sg[:, 0:1])
                vs = small.tile([G, 1], F32, tag="vs")
                nc.vector.tensor_sub(out=vs, in0=sg[:, 1:2], in1=t2)
                lnv = small.tile([G, 1], F32, tag="lnv")
                nc.scalar.activation(out=lnv, in_=vs, func=AF.Ln,
                                     bias=eps_t[0:G, :], scale=1.0)
                rstd = small.tile([G, 1], F32, tag="rstd")
                nc.scalar.activation(out=rstd, in_=lnv, func=AF.Exp, scale=-0.5)
                bcr = small.tile([G, 2], F32, tag="bcr")
                nc.gpsimd.tensor_copy(out=bcr[:, 0:1], in_=sg[:, 0:1])
                nc.gpsimd.tensor_copy(out=bcr[:, 1:2], in_=rstd)
                scs = []
                nbs = []
                for k in range(CK):
                    pbc_t = ps_w.tile([128, 512], F32, tag="wk")
                    pbc = pbc_t[:, 0:2]
                    nc.tensor.matmul(pbc, indb[:, k * 128:(k + 1) * 128], bcr,
                                     start=True, stop=True)
                    sc = small.tile([128, 1], F32, tag=f"sc{k}")
                    nc.vector.tensor_mul(out=sc, in0=pbc[:, 1:2], in1=gam[k])
                    t4 = small.tile([128, 1], F32, tag=f"t4{k}")
                    nc.vector.tensor_scalar_mul(out=t4, in0=pbc[:, 0:1],
                                                scalar1=sc)
                    nb = small.tile([128, 1], F32, tag=f"nb{k}")
                    nc.vector.tensor_sub(out=nb, in0=bet[k], in1=t4)
                    scs.append(sc)
                    nbs.append(nb)
                return scs, nbs

            def alloc_h(b):
                hs = [hpool.tile([128, N], BF, tag=f"h{k}", name=f"h{k}")
                      for k in range(CK)]
                h8 = [hpool.tile([128, 2, N], F8, tag=f"h8{p}", name=f"h8{p}")
                      for p in range(NP)]
                hps = [small.tile([128, 2], F32, tag=f"hp{k}", name=f"hp{k}")
                       for k in range(CK)]
                return hs, h8, hps

            def norm_act(xs, hctx, scs, nbs, k, half):
                """One [128,2048] normalize op on ACT (+accum partial)."""
                hs, h8, hps = hctx
                sl = slice(half * 2048, (half + 1) * 2048)
                nc.scalar.activation(
                    out=hs[k][:, sl], in_=xs[k][:, sl], func=AF.Identity,
                    bias=nbs[k], scale=scs[k],
                    accum_out=hps[k][:, half:half + 1])

            def h8_op(xs, hctx, scs, nbs, k, half):
                """Matching fp8 normalize on Pool (independent of norm_act)."""
                hs, h8, hps = hctx
                sl = slice(half * 2048, (half + 1) * 2048)
                nc.gpsimd.tensor_scalar(
                    out=h8[k // 2][:, k % 2, sl], in0=xs[k][:, sl],
                    scalar1=scs[k], scalar2=nbs[k], op0=OP.mult, op1=OP.add)

            def hsum_finish(hctx):
                hs, h8, hps = hctx
                hsums = []
                for k in range(CK):
                    t = small.tile([128, 1], BF, tag=f"hsum{k}")
                    nc.vector.tensor_add(out=t, in0=hps[k][:, 0:1],
                                         in1=hps[k][:, 1:2])
                    hsums.append(t)
                return hsums

            def spatial_chunk(b, hctx, Gs, s):
                hs, h8, hps = hctx
                # 4 transposes of h[:, s*128:(s+1)*128] into one bf16 psum tile
                pht = ps_tr.tile([128, 512], BF, tag="tr")
                for k in range(CK):
                    nc.tensor.transpose(
                        pht[:, k * 128:(k + 1) * 128],
                        hs[k][:, s * 128:(s + 1) * 128], ident)
                hT = mid.tile([128, 512], BF, tag="hT")
                # alternate evac engine so neither DVE nor ACT becomes the
                # spatial-loop straggler (DVE also runs bn_stats here)
                if s % 2 == 0:
                    nc.vector.tensor_copy(out=hT, in_=pht)
                else:
                    nc.scalar.copy(out=hT, in_=pht)
                # G[ck] += hT[:, ck].T @ hT
                for k in range(CK):
                    nc.tensor.matmul(Gs[k], hT[:, k * 128:(k + 1) * 128], hT,
                                     start=(s == 0), stop=(s == SP - 1))

            def vproj_t(b, hctx, t):
                hs, h8, hps = hctx
                vts = []
                for oc in range(CK):
                    pv = ps_w.tile([128, 512], F32, tag="wk")
                    for p in range(NP):
                        nc.tensor.matmul(
                            pv, wv8[p][:, :, oc * 128:(oc + 1) * 128],
                            h8[p][:, :, t * 512:(t + 1) * 512],
                            start=(p == 0), stop=(p == NP - 1), perf_mode=DR)
                    vt = vpool.tile([128, 512], BF, tag=f"v{oc}_{t}")
                    nc.scalar.mul(out=vt, in_=pv, mul=IWSC)
                    vts.append(vt)
                return vts

            def gram_finish(b, Gs, hsums):
                # evac G (bf16, symmetric)
                G_sb = []
                for k in range(CK):
                    t = gpool.tile([128, 512], BF, tag=f"G{k}")
                    nc.scalar.copy(out=t, in_=Gs[k])
                    G_sb.append(t)
                # qks row = hsum^T @ wqkT  -> [1, 2C]
                pq_t = ps_w.tile([128, 512], F32, tag="av")
                qks_sb = gpool.tile([1, 2 * C], BF, tag="qks")
                for half in range(2):
                    pq = pq_t[0:1, :]
                    for k in range(CK):
                        nc.tensor.matmul(
                            pq, hsums[k],
                            wqk[k][:, half * 512:(half + 1) * 512],
                            start=(k == 0), stop=(k == CK - 1))
                    nc.vector.tensor_copy(
                        out=qks_sb[:, half * 512:(half + 1) * 512], in_=pq)
                # ks2 = ks + N*bk
                ks2 = gpool.tile([1, C], BF, tag="ks2")
                nc.vector.scalar_tensor_tensor(
                    out=ks2, in0=bqkr[:, C:2 * C], scalar=float(N),
                    in1=qks_sb[:, C:2 * C], op0=OP.mult, op1=OP.add)
                # T = G @ Wk^T  (G symmetric: lhsT = G_sb[a][:, m-chunk])
                T_sb = []
                for m in range(CK):
                    pT = ps_w.tile([128, 512], F32, tag="wk")
                    for a in range(CK):
                        nc.tensor.matmul(
                            pT, G_sb[a][:, m * 128:(m + 1) * 128],
                            wqk[a][:, C:2 * C],
                            start=(a == 0), stop=(a == CK - 1))
                    t = gpool.tile([128, 512], BF, tag=f"T{m}")
                    nc.scalar.copy(out=t, in_=pT)
                    T_sb.append(t)
                # scores: per chunk ck, heads 2ck (even rows) / 2ck+1 (odd)
                # shares the "av" bank: qks (before) and cv (after) don't
                # overlap its lifetime
                SC = ps_w.tile([128, 256], F32, tag="av")
                for ck in range(CK):
                    for par in range(2):
                        hh = 2 * ck + par
                        hsl = slice(hh * 64, (hh + 1) * 64)
                        out_ap = SC[par * 64:(par + 1) * 64,
                                    ck * 64:(ck + 1) * 64]
                        tp = (0, par * 64)
                        nc.tensor.matmul(
                            out_ap, bqkr[:, hsl], ks2[:, hsl],
                            start=True, stop=False, tile_position=tp,
                            skip_group_check=True)
                        nc.tensor.matmul(
                            out_ap, qks_sb[:, hsl], bqkr[:, C + hh * 64:
                                                         C + (hh + 1) * 64],
                            start=False, stop=False, tile_position=tp,
                            skip_group_check=True)
                        for a in range(CK):
                            nc.tensor.matmul(
                                out_ap, wqk[a][:, hsl], T_sb[a][:, hsl],
                                start=False, stop=(a == CK - 1),
                                tile_position=tp, skip_group_check=True)
                return SC

            def softmax(b, SC):
                p_f = mid.tile([128, 256], F32, tag="pf", bufs=1)
                rs = mid.tile([128, CK], F32, tag="rs", bufs=1)
                for ck in range(CK):
                    for par in range(2):
                        rsl = slice(par * 64, (par + 1) * 64)
                        nc.scalar.activation(
                            out=p_f[rsl, ck * 64:(ck + 1) * 64],
                            in_=SC[rsl, ck * 64:(ck + 1) * 64],
                            func=AF.Exp, scale=scale,
                            accum_out=rs[rsl, ck:ck + 1])
                rv = mid.tile([128, CK], F32, tag="rv", bufs=1)
                nc.vector.reciprocal(out=rv, in_=rs)
                for ck in range(CK):
                    for par in range(2):
                        rsl = slice(par * 64, (par + 1) * 64)
                        nc.vector.tensor_scalar_mul(
                            out=att_bf[ck][rsl, par * 64:(par + 1) * 64],
                            in0=p_f[rsl, ck * 64:(ck + 1) * 64],
                            scalar1=rv[rsl, ck:ck + 1])

            def att_transpose(b):
                patt = ps_tr.tile([128, 512], BF, tag="tr")
                for ck in range(CK):
                    nc.tensor.transpose(
                        patt[:, ck * 128:(ck + 1) * 128], att_bf[ck], ident)
                attT = mid.tile([128, 512], BF, tag="attT", bufs=1)
                nc.vector.tensor_copy(out=attT, in_=patt)
                # cv = attT.T(!) applied to b_v: one matmul per chunk
                pcv_t = ps_w.tile([128, 512], F32, tag="av")
                for ck in range(CK):
                    nc.tensor.matmul(
                        pcv_t[:, ck:ck + 1],
                        attT[:, ck * 128:(ck + 1) * 128], bv_sb[ck],
                        start=True, stop=True, skip_group_check=True)
                cs4 = small.tile([128, CK], F32, tag="cs4")
                nc.vector.tensor_copy(out=cs4, in_=pcv_t[:, 0:CK])
                return attT, cs4

            def attv_t(b, attT, cs4, vsave, hv8, t, split_act=False):
                # att @ v for the 4 chunks of this t-block; psum rotates
                # through the (now free) G bank tags for a 4-deep pipeline
                for ck in range(CK):
                    pav = ps_g.tile([128, 512], F32, tag=f"G{ck}",
                                    name=f"pav{ck}")
                    nc.tensor.matmul(
                        pav, attT[:, ck * 128:(ck + 1) * 128],
                        vsave[t][ck], start=True, stop=True)
                    hv_ap = hv8[ck // 2][:, ck % 2, t * 512:(t + 1) * 512]
                    if split_act and ck % 2:
                        nc.scalar.activation(out=hv_ap, in_=pav,
                                             func=AF.Identity,
                                             bias=cs4[:, ck:ck + 1], scale=1.0)
                    else:
                        nc.vector.tensor_scalar_add(out=hv_ap, in0=pav,
                                                    scalar1=cs4[:, ck:ck + 1])

            def outproj_t(b, hv8, t):
                for oc in range(CK):
                    po = ps_w.tile([128, 512], F32,
                                   tag=("av" if oc % 2 else "wk"),
                                   name=f"po{oc}")
                    for p in range(NP):
                        nc.tensor.matmul(
                            po, wo8[p][:, :, oc * 128:(oc + 1) * 128],
                            hv8[p][:, :, t * 512:(t + 1) * 512],
                            start=(p == 0), stop=(p == NP - 1), perf_mode=DR)
                    xr = mid.tile([128, 512], F32, tag="xr", bufs=2)
                    nc.sync.dma_start(
                        out=xr,
                        in_=xf_d.ap()[b, oc * 128:(oc + 1) * 128,
                                      t * 512:(t + 1) * 512])
                    fin = mid.tile([128, 512], F32, tag="fin", bufs=2)
                    if oc % 2 == 0:
                        # P1: ACT (po/32 + bo) -> Pool (+x) -> gpsimd store
                        ot = mid.tile([128, 512], BF, tag="ot", bufs=2)
                        nc.scalar.activation(out=ot, in_=po, func=AF.Identity,
                                             bias=bo_sb[oc], scale=IWSC)
                        nc.gpsimd.tensor_add(out=fin, in0=xr, in1=ot)
                        dma_eng = nc.gpsimd
                    else:
                        # P2: DVE (po/32 + x) -> ACT (+bo) -> sync store
                        t1 = mid.tile([128, 512], F32, tag="t1", bufs=2)
                        nc.vector.scalar_tensor_tensor(
                            out=t1, in0=po, scalar=IWSC, in1=xr,
                            op0=OP.mult, op1=OP.add)
                        nc.scalar.activation(out=fin, in_=t1,
                                             func=AF.Identity,
                                             bias=bo_sb[oc], scale=1.0)
                        dma_eng = nc.sync
                    dma_eng.dma_start(
                        out=out_d.ap()[b, oc * 128:(oc + 1) * 128,
                                       t * 512:(t + 1) * 512],
                        in_=fin)

            # ---------------- pipelined emission ----------------
            # batch 0 prologue
            xs = load_x(0)
            for k in range(CK):
                for j in range(SUB):
                    emit_bn_stats(xs, k, j)
            scs, nbs = stats_finish(0)
            hctx = alloc_h(0)
            for k in range(CK):
                h8_op(xs, hctx, scs, nbs, k, 0)
            for k in range(CK):
                for half in range(2):
                    norm_act(xs, hctx, scs, nbs, k, half)
            for k in range(CK):
                h8_op(xs, hctx, scs, nbs, k, 1)
            hsums = hsum_finish(hctx)

            prev_hv8 = None  # previous batch's hv8 (out-proj deferred)
            for b in range(B):
                last = (b == B - 1)
                Gs = [ps_g.tile([128, 512], F32, tag=f"G{k}", name=f"G{k}")
                      for k in range(CK)]
                vsave = [None] * NT
                hv8 = [hpool.tile([128, 2, N], F8, tag=f"hv8{p}",
                                  name=f"hv8{p}")
                       for p in range(NP)]
                nxt_stats_ops = []
                if not last:
                    nxt_stats_ops = [(k, j) for k in range(CK)
                                     for j in range(SUB)]
                for s in range(SP):
                    if not last and s == 0:
                        xs_n = load_x(b + 1)
                    spatial_chunk(b, hctx, Gs, s)
                    if s % 4 == 3:
                        t = s // 4
                        if not last:
                            # last batch defers vproj past gram_finish so
                            # the softmax chain starts as early as possible
                            vsave[t] = vproj_t(b, hctx, t)
                        if prev_hv8 is not None:
                            outproj_t(b - 1, prev_hv8, t)
                    if not last and s >= 8:
                        # spread next batch's bn_stats: 1/s then 2/s
                        nops = 1 if s < 24 else 2
                        for _ in range(nops):
                            if nxt_stats_ops:
                                k, j = nxt_stats_ops.pop(0)
                                emit_bn_stats(xs_n, k, j)
                prev_hv8 = None
                SCp = gram_finish(b, Gs, hsums)
                softmax(b, SCp)
                if last:
                    for t in range(NT):
                        vsave[t] = vproj_t(b, hctx, t)
                else:
                    scs, nbs = stats_finish(b + 1)
                    hctx_n = alloc_h(b + 1)
                    # h8 half-0 first so the next spatial loop's vproj t=0
                    # has its input early
                    for k in range(CK):
                        h8_op(xs_n, hctx_n, scs, nbs, k, 0)
                    for k in range(CK):
                        for half in range(2):
                            norm_act(xs_n, hctx_n, scs, nbs, k, half)
                    for k in range(CK):
                        h8_op(xs_n, hctx_n, scs, nbs, k, 1)
                attT, cs4 = att_transpose(b)
                for t in range(NT):
                    attv_t(b, attT, cs4, vsave, hv8, t, split_act=last)
                if last:
                    for t in range(NT):
                        outproj_t(b, hv8, t)
                else:
                    prev_hv8 = hv8
                    hctx = hctx_n
                    hsums = hsum_finish(hctx)

    nc.compile()
    return nc


def make_indicators():
    ch = np.arange(C)
    grp = ch // (C // G)
    indf = np.zeros((C, G), np.float32)
    indf[ch, grp] = 1.0 / (C // G)
    indb = np.zeros((G, C), np.float32)
    indb[grp, ch] = 1.0
    return indf, indb


def prep_weights(w_qkv, b_qkv, w_out, b_out, gamma, beta):
    """Host-side weight layouts. Returns dict of per-core input tensors
    (excluding x)."""
    bf = ml_dtypes.bfloat16
    f8 = ml_dtypes.float8_e4m3
    w_qkv = np.asarray(w_qkv, np.float32)
    wqkT = np.ascontiguousarray(w_qkv[:2 * C].T).astype(bf)

    def pack_dr(wT):
        # wT [C, C] (contraction-major) -> [NP, 128, 2, C] fp8 scaled
        a = (np.asarray(wT, np.float32) * WSC).reshape(NP, 2, 128, C)
        return np.ascontiguousarray(a.transpose(0, 2, 1, 3)).astype(f8)

    wv8 = pack_dr(w_qkv[2 * C:].T)
    wo8 = pack_dr(np.asarray(w_out, np.float32).T)
    b_qkv = np.asarray(b_qkv, np.float32)
    indf, indb = make_indicators()
    return {
        "wqkT": wqkT, "wv8": wv8, "wo8": wo8,
        "bqkr": np.ascontiguousarray(b_qkv[:2 * C].reshape(1, -1)).astype(bf),
        "bv": np.ascontiguousarray(b_qkv[2 * C:].reshape(-1, 1)).astype(bf),
        "bo": np.ascontiguousarray(np.asarray(b_out, np.float32).reshape(-1, 1)),
        "gamma": np.ascontiguousarray(np.asarray(gamma, np.float32).reshape(-1, 1)),
        "beta": np.ascontiguousarray(np.asarray(beta, np.float32).reshape(-1, 1)),
        "indf": indf, "indb": indb,
    }


_PROGRAM = None


def _get_program():
    global _PROGRAM
    if _PROGRAM is None:
        _PROGRAM = build_program()
    return _PROGRAM


def kernel(x, gamma, beta, w_qkv, b_qkv, w_out, b_out):
    x = np.asarray(x)
    B, C_, H, W = x.shape
    N = H * W
    assert C_ == C and B == 16 and N == 4096
    nc = _get_program()

    bf = ml_dtypes.bfloat16
    wd = prep_weights(w_qkv, b_qkv, w_out, b_out, gamma, beta)
    xr = np.ascontiguousarray(x.reshape(B, C, N).astype(np.float32))
    xb = xr.astype(bf)

    bpc = B // N_CORES
    in_maps = []
    for c in range(N_CORES):
        m = {"xbf": xb[c * bpc:(c + 1) * bpc],
             "xf": xr[c * bpc:(c + 1) * bpc]}
        m.update(wd)
        in_maps.append(m)
    res = run_bass_kernel_spmd(nc, in_maps, core_ids=list(range(N_CORES)))
    out = np.concatenate([res.results[c]["out"] for c in range(N_CORES)],
                         axis=0)
    return out.reshape(B, C_, H, W).astype(np.float32)


# revision 15
# speedup vs baseline: 1.0021x; 1.0021x over previous
"""Trainium2 Bass kernel for nn_AttentionBlock (B=16, C=512, H=W=64, 8 heads).

Channel-attention block: GroupNorm(8 groups) -> 1x1 qkv -> scores over
channel dims (contract spatial N=4096) -> softmax -> att @ v -> 1x1 out
projection -> residual.

Sharding: data-parallel over batch, 2 per core, no collectives.

Key structure (vs a direct port of the reference):
  * scores are computed via the Gram matrix G = h @ h^T:
        S = Wq G Wk^T + qs x bk + bq x (ks + N*bk),  qs/ks = Wq/Wk @ rowsum(h)
    which costs C*C*N MACs (half of the q,k projections) and removes the
    [N, 2C] qk psum evacuation entirely.  G is built from PE transposes of
    h (spatial-on-partition tiles) accumulated over 32 spatial chunks.
  * v and out projections run in fp8 (e4m3) DoubleRow perf mode: weights
    are scaled by 32 host-side (avoids subnormals), inputs h/hv are stored
    fp8 in a paired-chunk layout [128, 2, N], and each matmul contracts
    256 channels at 2x throughput.  The 1/32 unscale folds into the psum
    evacuations.  (q,k stay bf16: softmax amplifies fp8 noise there.)
  * att is kept block-diagonal per channel-chunk ([128,128] tiles with two
    64x64 head blocks on the diagonal), so att@v and att@b_v are single
    full-width 128-contraction matmuls per (chunk, t-block).
  * x is loaded in bf16 (halves the startup DMA); the residual re-loads
    x in fp32 per 512-col block.
  * engine split: PE matmuls; ACT normalize+v/out/G/T evacs+exp; DVE
    bn_stats+hT/hv evacs+softmax small ops; Pool h8 normalize, residual
    adds and output stores.
"""

import numpy as np
import ml_dtypes

import concourse.bacc as bacc
import concourse.tile as tile
from concourse import mybir
from concourse.bass_utils import run_bass_kernel_spmd
from concourse.masks import make_identity

BF = mybir.dt.bfloat16
F8 = mybir.dt.float8e4
F32 = mybir.dt.float32
AX = mybir.AxisListType
OP = mybir.AluOpType
AF = mybir.ActivationFunctionType
DR = mybir.MatmulPerfMode.DoubleRow

C = 512
NH = 8
D = 64
G = 8
CK = C // 128   # 4 channel chunks
NP = 2          # chunk pairs for fp8 DoubleRow
EPS = 1e-5
N_CORES = 8
WSC = 32.0      # fp8 weight scale
IWSC = float(1.0 / WSC)


def build_program(B=2, N=4096, debug=False):
    SP = N // 128   # 32 spatial chunks
    NT = N // 512   # 8 t-blocks
    SUB = N // 512
    scale = float(1.0 / np.sqrt(D))

    nc = bacc.Bacc("TRN2", target_bir_lowering=False, debug=debug,
                   num_devices=N_CORES)

    xbf_d = nc.dram_tensor("xbf", [B, C, N], BF, kind="ExternalInput")
    xf_d = nc.dram_tensor("xf", [B, C, N], F32, kind="ExternalInput")
    wqkT_d = nc.dram_tensor("wqkT", [C, 2 * C], BF, kind="ExternalInput")
    wv8_d = nc.dram_tensor("wv8", [NP, 128, 2, C], F8, kind="ExternalInput")
    wo8_d = nc.dram_tensor("wo8", [NP, 128, 2, C], F8, kind="ExternalInput")
    bqk_d = nc.dram_tensor("bqkr", [1, 2 * C], BF, kind="ExternalInput")
    bv_d = nc.dram_tensor("bv", [C, 1], BF, kind="ExternalInput")
    bo_d = nc.dram_tensor("bo", [C, 1], F32, kind="ExternalInput")
    gam_d = nc.dram_tensor("gamma", [C, 1], F32, kind="ExternalInput")
    bet_d = nc.dram_tensor("beta", [C, 1], F32, kind="ExternalInput")
    indf_d = nc.dram_tensor("indf", [C, G], F32, kind="ExternalInput")
    indb_d = nc.dram_tensor("indb", [G, C], F32, kind="ExternalInput")
    out_d = nc.dram_tensor("out", [B, C, N], F32, kind="ExternalOutput")

    with tile.TileContext(nc) as tc:
        import contextlib
        ctx = contextlib.ExitStack()
        with ctx:
            persist = ctx.enter_context(tc.tile_pool(name="persist", bufs=1))
            xpool = ctx.enter_context(tc.tile_pool(name="xpool", bufs=1))
            hpool = ctx.enter_context(tc.tile_pool(name="hpool", bufs=1))
            vpool = ctx.enter_context(tc.tile_pool(name="vpool", bufs=1))
            gpool = ctx.enter_context(tc.tile_pool(name="gpool", bufs=1))
            mid = ctx.enter_context(tc.tile_pool(name="mid", bufs=3))
            small = ctx.enter_context(tc.tile_pool(name="small", bufs=1))
            # PSUM: G 4 banks (also att@v via tag rotation) + tr 2 +
            # wk 1 (vproj/stats/T) + av 1 (qks/scores/cv/outproj) = 8 banks
            ps_g = ctx.enter_context(
                tc.tile_pool(name="ps_g", bufs=1, space="PSUM"))
            ps_tr = ctx.enter_context(
                tc.tile_pool(name="ps_tr", bufs=2, space="PSUM"))
            ps_w = ctx.enter_context(
                tc.tile_pool(name="ps_w", bufs=1, space="PSUM"))

            # ---- persistent weights / constants ----
            wqk = []
            for k in range(CK):
                t = persist.tile([128, 2 * C], BF, tag=f"wqk{k}")
                nc.gpsimd.dma_start(out=t, in_=wqkT_d.ap()[k * 128:(k + 1) * 128, :])
                wqk.append(t)
            wv8 = []
            wo8 = []
            for p in range(NP):
                t = persist.tile([128, 2, C], F8, tag=f"wv8{p}")
                nc.gpsimd.dma_start(out=t, in_=wv8_d.ap()[p])
                wv8.append(t)
                t = persist.tile([128, 2, C], F8, tag=f"wo8{p}")
                nc.gpsimd.dma_start(out=t, in_=wo8_d.ap()[p])
                wo8.append(t)
            bqkr = persist.tile([1, 2 * C], BF, tag="bqkr")
            nc.gpsimd.dma_start(out=bqkr, in_=bqk_d.ap())
            bv_sb = []
            bo_sb = []
            gam = []
            bet = []
            indf = []
            for k in range(CK):
                t = persist.tile([128, 1], BF, tag=f"bv{k}")
                nc.gpsimd.dma_start(out=t, in_=bv_d.ap()[k * 128:(k + 1) * 128, :])
                bv_sb.append(t)
                t = persist.tile([128, 1], F32, tag=f"bo{k}")
                nc.gpsimd.dma_start(out=t, in_=bo_d.ap()[k * 128:(k + 1) * 128, :])
                bo_sb.append(t)
                t = persist.tile([128, 1], F32, tag=f"gam{k}")
                nc.gpsimd.dma_start(out=t, in_=gam_d.ap()[k * 128:(k + 1) * 128, :])
                gam.append(t)
                t = persist.tile([128, 1], F32, tag=f"bet{k}")
                nc.gpsimd.dma_start(out=t, in_=bet_d.ap()[k * 128:(k + 1) * 128, :])
                bet.append(t)
                t = persist.tile([128, G], F32, tag=f"indf{k}")
                nc.gpsimd.dma_start(out=t, in_=indf_d.ap()[k * 128:(k + 1) * 128, :])
                indf.append(t)
            indb = persist.tile([G, C], F32, tag="indb")
            nc.gpsimd.dma_start(out=indb, in_=indb_d.ap())
            ident = persist.tile([128, 128], BF, tag="ident")
            make_identity(nc, ident)
            eps_t = persist.tile([128, 1], F32, tag="eps")
            nc.gpsimd.memset(eps_t, EPS)
            # block-diag att tiles: off-diagonal quadrants stay zero forever
            att_bf = []
            for k in range(CK):
                t = persist.tile([128, 128], BF, tag=f"attb{k}")
                nc.gpsimd.memset(t, 0.0)
                att_bf.append(t)

            # ---------------- phase helpers ----------------
            def load_x(b):
                xs = []
                for k in range(CK):
                    t = xpool.tile([128, N], BF, tag=f"x{k}")
                    for q2 in range(0, N, 2048):
                        nc.sync.dma_start(
                            out=t[:, q2:q2 + 2048],
                            in_=xbf_d.ap()[b, k * 128:(k + 1) * 128,
                                           q2:q2 + 2048])
                    xs.append(t)
                return xs

            def emit_bn_stats(xs, k, j):
                # one bn_stats op on a 512-col block; st tile per chunk
                if j == 0:
                    st = small.tile([128, SUB, 6], F32, tag=f"st{k}")
                    emit_bn_stats.st[k] = st
                st = emit_bn_stats.st[k]
                nc.vector.bn_stats(out=st[:, j, :],
                                   in_=xs[k][:, j * 512:(j + 1) * 512])
            emit_bn_stats.st = [None] * CK

            def stats_finish(b):
                """bn_aggr + group reduce -> per-channel sc/nb coeffs."""
                mvs = []
                rhs2s = []
                for k in range(CK):
                    mv = small.tile([128, 2], F32, tag=f"mv{k}")
                    nc.vector.bn_aggr(out=mv, in_=emit_bn_stats.st[k])
                    mvs.append(mv)
                    r2 = small.tile([128, 2], F32, tag=f"r2{k}")
                    nc.gpsimd.tensor_copy(out=r2[:, 0:1], in_=mv[:, 0:1])
                    nc.vector.scalar_tensor_tensor(
                        out=r2[:, 1:2], in0=mv[:, 0:1],
                        scalar=mv[:, 0:1], in1=mv[:, 1:2],
                        op0=OP.mult, op1=OP.add)
                    rhs2s.append(r2)
                pg_t = ps_w.tile([128, 512], F32, tag="wk")
                pg = pg_t[0:G, 0:2]
                for k in range(CK):
                    nc.tensor.matmul(pg, indf[k], rhs2s[k],
                                     start=(k == 0), stop=(k == CK - 1))
                sg = small.tile([G, 2], F32, tag="sg")
                nc.vector.tensor_copy(out=sg, in_=pg)
                t2 = small.tile([G, 1], F32, tag="t2")
                nc.vector.tensor_mul(out=t2, in0=sg[:, 0:1], in1=sg[:, 0:1])
                vs = small.tile([G, 1], F32, tag="vs")
                nc.vector.tensor_sub(out=vs, in0=sg[:, 1:2], in1=t2)
                lnv = small.tile([G, 1], F32, tag="lnv")
                nc.scalar.activation(out=lnv, in_=vs, func=AF.Ln,
                                     bias=eps_t[0:G, :], scale=1.0)
                rstd = small.tile([G, 1], F32, tag="rstd")
                nc.scalar.activation(out=rstd, in_=lnv, func=AF.Exp, scale=-0.5)
                bcr = small.tile([G, 2], F32, tag="bcr")
                nc.gpsimd.tensor_copy(out=bcr[:, 0:1], in_=sg[:, 0:1])
                nc.gpsimd.tensor_copy(out=bcr[:, 1:2], in_=rstd)
                scs = []
                nbs = []
                for k in range(CK):
                    pbc_t = ps_w.tile([128, 512], F32, tag="wk")
                    pbc = pbc_t[:, 0:2]
                    nc.tensor.matmul(pbc, indb[:, k * 128:(k + 1) * 128], bcr,
                                     start=True, stop=True)
                    sc = small.tile([128, 1], F32, tag=f"sc{k}")
                    nc.vector.tensor_mul(out=sc, in0=pbc[:, 1:2], in1=gam[k])
                    t4 = small.tile([128, 1], F32, tag=f"t4{k}")
                    nc.vector.tensor_scalar_mul(out=t4, in0=pbc[:, 0:1],
                                                scalar1=sc)
                    nb = small.tile([128, 1], F32, tag=f"nb{k}")
                    nc.vector.tensor_sub(out=nb, in0=bet[k], in1=t4)
                    scs.append(sc)
                    nbs.append(nb)
                return scs, nbs

            def alloc_h(b):
                hs = [hpool.tile([128, N], BF, tag=f"h{k}", name=f"h{k}")
                      for k in range(CK)]
                h8 = [hpool.tile([128, 2, N], F8, tag=f"h8{p}", name=f"h8{p}")
                      for p in range(NP)]
                hps = [small.tile([128, 2], F32, tag=f"hp{k}", name=f"hp{k}")
                       for k in range(CK)]
                return hs, h8, hps

            def norm_act(xs, hctx, scs, nbs, k, half):
                """One [128,2048] normalize op on ACT (+accum partial)."""
                hs, h8, hps = hctx
                sl = slice(half * 2048, (half + 1) * 2048)
                nc.scalar.activation(
                    out=hs[k][:, sl], in_=xs[k][:, sl], func=AF.Identity,
                    bias=nbs[k], scale=scs[k],
                    accum_out=hps[k][:, half:half + 1])

            def h8_op(xs, hctx, scs, nbs, k, half):
                """Matching fp8 normalize on Pool (independent of norm_act)."""
                hs, h8, hps = hctx
                sl = slice(half * 2048, (half + 1) * 2048)
                nc.gpsimd.tensor_scalar(
                    out=h8[k // 2][:, k % 2, sl], in0=xs[k][:, sl],
                    scalar1=scs[k], scalar2=nbs[k], op0=OP.mult, op1=OP.add)

            def hsum_finish(hctx):
                hs, h8, hps = hctx
                hsums = []
                for k in range(CK):
                    t = small.tile([128, 1], BF, tag=f"hsum{k}")
                    nc.vector.tensor_add(out=t, in0=hps[k][:, 0:1],
                                         in1=hps[k][:, 1:2])
                    hsums.append(t)
                return hsums

            def transpose_chunk(b, hctx, s):
                hs, h8, hps = hctx
                # 4 transposes of h[:, s*128:(s+1)*128] into one bf16 psum tile
                pht = ps_tr.tile([128, 512], BF, tag="tr")
                for k in range(CK):
                    nc.tensor.transpose(
                        pht[:, k * 128:(k + 1) * 128],
                        hs[k][:, s * 128:(s + 1) * 128], ident)
                hT = mid.tile([128, 512], BF, tag="hT", bufs=4)
                # alternate evac engine so neither DVE nor ACT becomes the
                # spatial-loop straggler (DVE also runs bn_stats here)
                if s % 2 == 0:
                    nc.vector.tensor_copy(out=hT, in_=pht)
                else:
                    nc.scalar.copy(out=hT, in_=pht)
                return hT

            def gram_chunk(b, Gs, hT, s):
                # G[ck] += hT[:, ck].T @ hT
                for k in range(CK):
                    nc.tensor.matmul(Gs[k], hT[:, k * 128:(k + 1) * 128], hT,
                                     start=(s == 0), stop=(s == SP - 1))

            def vproj_t(b, hctx, t):
                hs, h8, hps = hctx
                vts = []
                for oc in range(CK):
                    pv = ps_w.tile([128, 512], F32,
                                   tag=("av" if oc % 2 else "wk"), name="pv")
                    for p in range(NP):
                        nc.tensor.matmul(
                            pv, wv8[p][:, :, oc * 128:(oc + 1) * 128],
                            h8[p][:, :, t * 512:(t + 1) * 512],
                            start=(p == 0), stop=(p == NP - 1), perf_mode=DR)
                    vt = vpool.tile([128, 512], BF, tag=f"v{oc}_{t}")
                    nc.scalar.mul(out=vt, in_=pv, mul=IWSC)
                    vts.append(vt)
                return vts

            def gram_finish(b, Gs, hsums):
                # qks row = hsum^T @ wqkT  -> [1, 2C]  (independent of G)
                pq_t = ps_w.tile([128, 512], F32, tag="av")
                qks_sb = gpool.tile([1, 2 * C], BF, tag="qks")
                for half in range(2):
                    pq = pq_t[0:1, :]
                    for k in range(CK):
                        nc.tensor.matmul(
                            pq, hsums[k],
                            wqk[k][:, half * 512:(half + 1) * 512],
                            start=(k == 0), stop=(k == CK - 1))
                    nc.vector.tensor_copy(
                        out=qks_sb[:, half * 512:(half + 1) * 512], in_=pq)
                # evac G (bf16, symmetric)
                G_sb = []
                for k in range(CK):
                    t = gpool.tile([128, 512], BF, tag=f"G{k}")
                    nc.scalar.copy(out=t, in_=Gs[k])
                    G_sb.append(t)
                # ks2 = ks + N*bk
                ks2 = gpool.tile([1, C], BF, tag="ks2")
                nc.vector.scalar_tensor_tensor(
                    out=ks2, in0=bqkr[:, C:2 * C], scalar=float(N),
                    in1=qks_sb[:, C:2 * C], op0=OP.mult, op1=OP.add)
                # T = G @ Wk^T  (G symmetric: lhsT = G_sb[a][:, m-chunk])
                T_sb = []
                for m in range(CK):
                    pT = ps_w.tile([128, 512], F32, tag="wk")
                    for a in range(CK):
                        nc.tensor.matmul(
                            pT, G_sb[a][:, m * 128:(m + 1) * 128],
                            wqk[a][:, C:2 * C],
                            start=(a == 0), stop=(a == CK - 1))
                    t = gpool.tile([128, 512], BF, tag=f"T{m}")
                    nc.scalar.copy(out=t, in_=pT)
                    T_sb.append(t)
                # scores: per chunk ck, heads 2ck (even rows) / 2ck+1 (odd)
                # shares the "av" bank: qks (before) and cv (after) don't
                # overlap its lifetime
                SC = ps_w.tile([128, 256], F32, tag="av")
                for ck in range(CK):
                    for par in range(2):
                        hh = 2 * ck + par
                        hsl = slice(hh * 64, (hh + 1) * 64)
                        out_ap = SC[par * 64:(par + 1) * 64,
                                    ck * 64:(ck + 1) * 64]
                        tp = (0, par * 64)
                        nc.tensor.matmul(
                            out_ap, bqkr[:, hsl], ks2[:, hsl],
                            start=True, stop=False, tile_position=tp,
                            skip_group_check=True)
                        nc.tensor.matmul(
                            out_ap, qks_sb[:, hsl], bqkr[:, C + hh * 64:
                                                         C + (hh + 1) * 64],
                            start=False, stop=False, tile_position=tp,
                            skip_group_check=True)
                        for a in range(CK):
                            nc.tensor.matmul(
                                out_ap, wqk[a][:, hsl], T_sb[a][:, hsl],
                                start=False, stop=(a == CK - 1),
                                tile_position=tp, skip_group_check=True)
                return SC

            def softmax(b, SC):
                p_f = mid.tile([128, 256], F32, tag="pf", bufs=1)
                rs = mid.tile([128, CK], F32, tag="rs", bufs=1)
                for ck in range(CK):
                    for par in range(2):
                        rsl = slice(par * 64, (par + 1) * 64)
                        nc.scalar.activation(
                            out=p_f[rsl, ck * 64:(ck + 1) * 64],
                            in_=SC[rsl, ck * 64:(ck + 1) * 64],
                            func=AF.Exp, scale=scale,
                            accum_out=rs[rsl, ck:ck + 1])
                rv = mid.tile([128, CK], F32, tag="rv", bufs=1)
                nc.vector.reciprocal(out=rv, in_=rs)
                for ck in range(CK):
                    for par in range(2):
                        rsl = slice(par * 64, (par + 1) * 64)
                        nc.vector.tensor_scalar_mul(
                            out=att_bf[ck][rsl, par * 64:(par + 1) * 64],
                            in0=p_f[rsl, ck * 64:(ck + 1) * 64],
                            scalar1=rv[rsl, ck:ck + 1])

            def att_transpose(b):
                patt = ps_tr.tile([128, 512], BF, tag="tr")
                for ck in range(CK):
                    nc.tensor.transpose(
                        patt[:, ck * 128:(ck + 1) * 128], att_bf[ck], ident)
                attT = mid.tile([128, 512], BF, tag="attT", bufs=1)
                nc.vector.tensor_copy(out=attT, in_=patt)
                # cv = attT.T(!) applied to b_v: one matmul per chunk
                pcv_t = ps_w.tile([128, 512], F32, tag="av")
                for ck in range(CK):
                    nc.tensor.matmul(
                        pcv_t[:, ck:ck + 1],
                        attT[:, ck * 128:(ck + 1) * 128], bv_sb[ck],
                        start=True, stop=True, skip_group_check=True)
                cs4 = small.tile([128, CK], F32, tag="cs4")
                nc.vector.tensor_copy(out=cs4, in_=pcv_t[:, 0:CK])
                return attT, cs4

            def attv_t(b, attT, cs4, vsave, hv8, t, split_act=False):
                # att @ v for the 4 chunks of this t-block; psum rotates
                # through the (now free) G bank tags for a 4-deep pipeline
                for ck in range(CK):
                    pav = ps_g.tile([128, 512], F32, tag=f"G{ck}",
                                    name=f"pav{ck}")
                    nc.tensor.matmul(
                        pav, attT[:, ck * 128:(ck + 1) * 128],
                        vsave[t][ck], start=True, stop=True)
                    hv_ap = hv8[ck // 2][:, ck % 2, t * 512:(t + 1) * 512]
                    if split_act and ck % 2:
                        nc.scalar.activation(out=hv_ap, in_=pav,
                                             func=AF.Identity,
                                             bias=cs4[:, ck:ck + 1], scale=1.0)
                    else:
                        nc.vector.tensor_scalar_add(out=hv_ap, in0=pav,
                                                    scalar1=cs4[:, ck:ck + 1])

            def outproj_t(b, hv8, t):
                for oc in range(CK):
                    po = ps_w.tile([128, 512], F32,
                                   tag=("av" if oc % 2 else "wk"),
                                   name=f"po{oc}")
                    for p in range(NP):
                        nc.tensor.matmul(
                            po, wo8[p][:, :, oc * 128:(oc + 1) * 128],
                            hv8[p][:, :, t * 512:(t + 1) * 512],
                            start=(p == 0), stop=(p == NP - 1), perf_mode=DR)
                    xr = mid.tile([128, 512], F32, tag="xr", bufs=2)
                    nc.sync.dma_start(
                        out=xr,
                        in_=xf_d.ap()[b, oc * 128:(oc + 1) * 128,
                                      t * 512:(t + 1) * 512])
                    fin = mid.tile([128, 512], F32, tag="fin", bufs=2)
                    if oc % 2 == 0:
                        # P1: ACT (po/32 + bo) -> Pool (+x) -> gpsimd store
                        ot = mid.tile([128, 512], BF, tag="ot", bufs=2)
                        nc.scalar.activation(out=ot, in_=po, func=AF.Identity,
                                             bias=bo_sb[oc], scale=IWSC)
                        nc.gpsimd.tensor_add(out=fin, in0=xr, in1=ot)
                        dma_eng = nc.gpsimd
                    else:
                        # P2: DVE (po/32 + x) -> ACT (+bo) -> sync store
                        t1 = mid.tile([128, 512], F32, tag="t1", bufs=2)
                        nc.vector.scalar_tensor_tensor(
                            out=t1, in0=po, scalar=IWSC, in1=xr,
                            op0=OP.mult, op1=OP.add)
                        nc.scalar.activation(out=fin, in_=t1,
                                             func=AF.Identity,
                                             bias=bo_sb[oc], scale=1.0)
                        dma_eng = nc.sync
                    dma_eng.dma_start(
                        out=out_d.ap()[b, oc * 128:(oc + 1) * 128,
                                       t * 512:(t + 1) * 512],
                        in_=fin)

            # ---------------- pipelined emission ----------------
            # batch 0 prologue
            xs = load_x(0)
            for k in range(CK):
                for j in range(SUB):
                    emit_bn_stats(xs, k, j)
            scs, nbs = stats_finish(0)
            hctx = alloc_h(0)
            for k in range(CK):
                h8_op(xs, hctx, scs, nbs, k, 0)
            for half in range(2):
                for k in range(CK):
                    norm_act(xs, hctx, scs, nbs, k, half)
            for k in range(CK):
                h8_op(xs, hctx, scs, nbs, k, 1)
            hsums = hsum_finish(hctx)

            prev_hv8 = None  # previous batch's hv8 (out-proj deferred)
            for b in range(B):
                last = (b == B - 1)
                Gs = [ps_g.tile([128, 512], F32, tag=f"G{k}", name=f"G{k}")
                      for k in range(CK)]
                vsave = [None] * NT
                hv8 = [hpool.tile([128, 2, N], F8, tag=f"hv8{p}",
                                  name=f"hv8{p}")
                       for p in range(NP)]
                nxt_stats_ops = []
                if not last:
                    nxt_stats_ops = [(k, j) for k in range(CK)
                                     for j in range(SUB)]
                GLAG = 2  # transposes run 2 chunks ahead of G matmuls
                hT_q = []
                for s in range(SP + GLAG):
                    if not last and s == 0:
                        xs_n = load_x(b + 1)
                    if s < SP:
                        hT_q.append(transpose_chunk(b, hctx, s))
                    if s >= GLAG:
                        gram_chunk(b, Gs, hT_q[s - GLAG], s - GLAG)
                    if s % 4 == 3:
                        t = s // 4
                        if not last:
                            # last batch defers vproj past gram_finish so
                            # the softmax chain starts as early as possible
                            vsave[t] = vproj_t(b, hctx, t)
                        if prev_hv8 is not None and t < NT:
                            outproj_t(b - 1, prev_hv8, t)
                    if not last and s >= 8:
                        # spread next batch's bn_stats: 1/s then 2/s
                        nops = 1 if s < 24 else 2
                        for _ in range(nops):
                            if nxt_stats_ops:
                                k, j = nxt_stats_ops.pop(0)
                                emit_bn_stats(xs_n, k, j)
                prev_hv8 = None
                SCp = gram_finish(b, Gs, hsums)
                softmax(b, SCp)
                if last:
                    for t in range(NT):
                        vsave[t] = vproj_t(b, hctx, t)
                else:
                    scs, nbs = stats_finish(b + 1)
                    hctx_n = alloc_h(b + 1)
                    # h8 half-0 first so the next spatial loop's vproj t=0
                    # has its input early
                    for k in range(CK):
                        h8_op(xs_n, hctx_n, scs, nbs, k, 0)
                    for half in range(2):
                        for k in range(CK):
                            norm_act(xs_n, hctx_n, scs, nbs, k, half)
                    for k in range(CK):
                        h8_op(xs_n, hctx_n, scs, nbs, k, 1)
                attT, cs4 = att_transpose(b)
                for t in range(NT):
                    attv_t(b, attT, cs4, vsave, hv8, t, split_act=last)
                if last:
                    for t in range(NT):
                        outproj_t(b, hv8, t)
                else:
                    prev_hv8 = hv8
                    hctx = hctx_n
                    hsums = hsum_finish(hctx)

    nc.compile()
    return nc


def make_indicators():
    ch = np.arange(C)
    grp = ch // (C // G)
    indf = np.zeros((C, G), np.float32)
    indf[ch, grp] = 1.0 / (C // G)
    indb = np.zeros((G, C), np.float32)
    indb[grp, ch] = 1.0
    return indf, indb


def prep_weights(w_qkv, b_qkv, w_out, b_out, gamma, beta):
    """Host-side weight layouts. Returns dict of per-core input tensors
    (excluding x)."""
    bf = ml_dtypes.bfloat16
    f8 = ml_dtypes.float8_e4m3
    w_qkv = np.asarray(w_qkv, np.float32)
    wqkT = np.ascontiguousarray(w_qkv[:2 * C].T).astype(bf)

    def pack_dr(wT):
        # wT [C, C] (contraction-major) -> [NP, 128, 2, C] fp8 scaled
        a = (np.asarray(wT, np.float32) * WSC).reshape(NP, 2, 128, C)
        return np.ascontiguousarray(a.transpose(0, 2, 1, 3)).astype(f8)

    wv8 = pack_dr(w_qkv[2 * C:].T)
    wo8 = pack_dr(np.asarray(w_out, np.float32).T)
    b_qkv = np.asarray(b_qkv, np.float32)
    indf, indb = make_indicators()
    return {
        "wqkT": wqkT, "wv8": wv8, "wo8": wo8,
        "bqkr": np.ascontiguousarray(b_qkv[:2 * C].reshape(1, -1)).astype(bf),
        "bv": np.ascontiguousarray(b_qkv[2 * C:].reshape(-1, 1)).astype(bf),
        "bo": np.ascontiguousarray(np.asarray(b_out, np.float32).reshape(-1, 1)),
        "gamma": np.ascontiguousarray(np.asarray(gamma, np.float32).reshape(-1, 1)),
        "beta": np.ascontiguousarray(np.asarray(beta, np.float32).reshape(-1, 1)),
        "indf": indf, "indb": indb,
    }


_PROGRAM = None


def _get_program():
    global _PROGRAM
    if _PROGRAM is None:
        _PROGRAM = build_program()
    return _PROGRAM


def kernel(x, gamma, beta, w_qkv, b_qkv, w_out, b_out):
    x = np.asarray(x)
    B, C_, H, W = x.shape
    N = H * W
    assert C_ == C and B == 16 and N == 4096
    nc = _get_program()

    bf = ml_dtypes.bfloat16
    wd = prep_weights(w_qkv, b_qkv, w_out, b_out, gamma, beta)
    xr = np.ascontiguousarray(x.reshape(B, C, N).astype(np.float32))
    xb = xr.astype(bf)

    bpc = B // N_CORES
    in_maps = []
    for c in range(N_CORES):
        m = {"xbf": xb[c * bpc:(c + 1) * bpc],
             "xf": xr[c * bpc:(c + 1) * bpc]}
        m.update(wd)
        in_maps.append(m)
    res = run_bass_kernel_spmd(nc, in_maps, core_ids=list(range(N_CORES)))
    out = np.concatenate([res.results[c]["out"] for c in range(N_CORES)],
                         axis=0)
    return out.reshape(B, C_, H, W).astype(np.float32)


# revision 17
# speedup vs baseline: 1.0311x; 1.0289x over previous
"""Trainium2 Bass kernel for nn_AttentionBlock (B=16, C=512, H=W=64, 8 heads).

Channel-attention block: GroupNorm(8 groups) -> 1x1 qkv -> scores over
channel dims (contract spatial N=4096) -> softmax -> att @ v -> 1x1 out
projection -> residual.

Sharding: data-parallel over batch, 2 per core, no collectives.

Key structure (vs a direct port of the reference):
  * scores are computed via the Gram matrix G = h @ h^T:
        S = Wq G Wk^T + qs x bk + bq x (ks + N*bk),  qs/ks = Wq/Wk @ rowsum(h)
    which costs C*C*N MACs (half of the q,k projections) and removes the
    [N, 2C] qk psum evacuation entirely.  G is built from PE transposes of
    h (spatial-on-partition tiles) accumulated over 32 spatial chunks.
  * v and out projections run in fp8 (e4m3) DoubleRow perf mode: weights
    are scaled by 32 host-side (avoids subnormals), inputs h/hv are stored
    fp8 in a paired-chunk layout [128, 2, N], and each matmul contracts
    256 channels at 2x throughput.  The 1/32 unscale folds into the psum
    evacuations.  (q,k stay bf16: softmax amplifies fp8 noise there.)
  * att is kept block-diagonal per channel-chunk ([128,128] tiles with two
    64x64 head blocks on the diagonal), so att@v and att@b_v are single
    full-width 128-contraction matmuls per (chunk, t-block).
  * x is loaded in bf16 (halves the startup DMA); the residual re-loads
    x in fp32 per 512-col block.
  * engine split: PE matmuls; ACT normalize+v/out/G/T evacs+exp; DVE
    bn_stats+hT/hv evacs+softmax small ops; Pool h8 normalize, residual
    adds and output stores.
"""

import numpy as np
import ml_dtypes

import concourse.bacc as bacc
import concourse.tile as tile
from concourse import mybir
from concourse.bass_utils import run_bass_kernel_spmd
from concourse.masks import make_identity

BF = mybir.dt.bfloat16
F8 = mybir.dt.float8e4
F32 = mybir.dt.float32
AX = mybir.AxisListType
OP = mybir.AluOpType
AF = mybir.ActivationFunctionType
DR = mybir.MatmulPerfMode.DoubleRow

C = 512
NH = 8
D = 64
G = 8
CK = C // 128   # 4 channel chunks
NP = 2          # chunk pairs for fp8 DoubleRow
EPS = 1e-5
N_CORES = 8
WSC = 32.0      # fp8 weight scale
IWSC = float(1.0 / WSC)


def build_program(B=2, N=4096, debug=False):
    SP = N // 128   # 32 spatial chunks
    NT = N // 512   # 8 t-blocks
    SUB = N // 512
    scale = float(1.0 / np.sqrt(D))

    nc = bacc.Bacc("TRN2", target_bir_lowering=False, debug=debug,
                   num_devices=N_CORES)

    xbf_d = nc.dram_tensor("xbf", [B, C, N], BF, kind="ExternalInput")
    xf_d = nc.dram_tensor("xf", [B, C, N], F32, kind="ExternalInput")
    wqkT_d = nc.dram_tensor("wqkT", [C, 2 * C], BF, kind="ExternalInput")
    wv8_d = nc.dram_tensor("wv8", [NP, 128, 2, C], F8, kind="ExternalInput")
    wo8_d = nc.dram_tensor("wo8", [NP, 128, 2, C], F8, kind="ExternalInput")
    bqk_d = nc.dram_tensor("bqkr", [1, 2 * C], BF, kind="ExternalInput")
    bv_d = nc.dram_tensor("bv", [C, 1], BF, kind="ExternalInput")
    bo_d = nc.dram_tensor("bo", [C, 1], F32, kind="ExternalInput")
    gam_d = nc.dram_tensor("gamma", [C, 1], F32, kind="ExternalInput")
    bet_d = nc.dram_tensor("beta", [C, 1], F32, kind="ExternalInput")
    indf_d = nc.dram_tensor("indf", [C, G], F32, kind="ExternalInput")
    indb_d = nc.dram_tensor("indb", [G, C], F32, kind="ExternalInput")
    out_d = nc.dram_tensor("out", [B, C, N], F32, kind="ExternalOutput")

    with tile.TileContext(nc) as tc:
        import contextlib
        ctx = contextlib.ExitStack()
        with ctx:
            persist = ctx.enter_context(tc.tile_pool(name="persist", bufs=1))
            xpool = ctx.enter_context(tc.tile_pool(name="xpool", bufs=1))
            hpool = ctx.enter_context(tc.tile_pool(name="hpool", bufs=1))
            vpool = ctx.enter_context(tc.tile_pool(name="vpool", bufs=1))
            gpool = ctx.enter_context(tc.tile_pool(name="gpool", bufs=1))
            mid = ctx.enter_context(tc.tile_pool(name="mid", bufs=3))
            small = ctx.enter_context(tc.tile_pool(name="small", bufs=1))
            # PSUM: G 4 banks (also att@v via tag rotation) + tr 2 +
            # wk 1 (vproj/stats/T) + av 1 (qks/scores/cv/outproj) = 8 banks
            ps_g = ctx.enter_context(
                tc.tile_pool(name="ps_g", bufs=1, space="PSUM"))
            ps_tr = ctx.enter_context(
                tc.tile_pool(name="ps_tr", bufs=2, space="PSUM"))
            ps_w = ctx.enter_context(
                tc.tile_pool(name="ps_w", bufs=1, space="PSUM"))

            # ---- persistent weights / constants ----
            wqk = []
            for k in range(CK):
                t = persist.tile([128, 2 * C], BF, tag=f"wqk{k}")
                nc.gpsimd.dma_start(out=t, in_=wqkT_d.ap()[k * 128:(k + 1) * 128, :])
                wqk.append(t)
            wv8 = []
            wo8 = []
            for p in range(NP):
                t = persist.tile([128, 2, C], F8, tag=f"wv8{p}")
                nc.gpsimd.dma_start(out=t, in_=wv8_d.ap()[p])
                wv8.append(t)
                t = persist.tile([128, 2, C], F8, tag=f"wo8{p}")
                nc.gpsimd.dma_start(out=t, in_=wo8_d.ap()[p])
                wo8.append(t)
            bqkr = persist.tile([1, 2 * C], BF, tag="bqkr")
            nc.gpsimd.dma_start(out=bqkr, in_=bqk_d.ap())
            bv_sb = []
            bo_sb = []
            gam = []
            bet = []
            indf = []
            for k in range(CK):
                t = persist.tile([128, 1], BF, tag=f"bv{k}")
                nc.gpsimd.dma_start(out=t, in_=bv_d.ap()[k * 128:(k + 1) * 128, :])
                bv_sb.append(t)
                t = persist.tile([128, 1], F32, tag=f"bo{k}")
                nc.gpsimd.dma_start(out=t, in_=bo_d.ap()[k * 128:(k + 1) * 128, :])
                bo_sb.append(t)
                t = persist.tile([128, 1], F32, tag=f"gam{k}")
                nc.gpsimd.dma_start(out=t, in_=gam_d.ap()[k * 128:(k + 1) * 128, :])
                gam.append(t)
                t = persist.tile([128, 1], F32, tag=f"bet{k}")
                nc.gpsimd.dma_start(out=t, in_=bet_d.ap()[k * 128:(k + 1) * 128, :])
                bet.append(t)
                t = persist.tile([128, G], F32, tag=f"indf{k}")
                nc.gpsimd.dma_start(out=t, in_=indf_d.ap()[k * 128:(k + 1) * 128, :])
                indf.append(t)
            indb = persist.tile([G, C], F32, tag="indb")
            nc.gpsimd.dma_start(out=indb, in_=indb_d.ap())
            ident = persist.tile([128, 128], BF, tag="ident")
            make_identity(nc, ident)
            eps_t = persist.tile([128, 1], F32, tag="eps")
            nc.gpsimd.memset(eps_t, EPS)
            # block-diag att tiles: off-diagonal quadrants stay zero forever
            att_bf = []
            for k in range(CK):
                t = persist.tile([128, 128], BF, tag=f"attb{k}")
                nc.gpsimd.memset(t, 0.0)
                att_bf.append(t)

            # ---------------- phase helpers ----------------
            def load_x(b):
                xs = []
                for k in range(CK):
                    t = xpool.tile([128, N], BF, tag=f"x{k}")
                    for q2 in range(0, N, 2048):
                        nc.sync.dma_start(
                            out=t[:, q2:q2 + 2048],
                            in_=xbf_d.ap()[b, k * 128:(k + 1) * 128,
                                           q2:q2 + 2048])
                    xs.append(t)
                return xs

            def emit_bn_stats(xs, k, j):
                # one bn_stats op on a 512-col block; st tile per chunk
                if j == 0:
                    st = small.tile([128, SUB, 6], F32, tag=f"st{k}")
                    emit_bn_stats.st[k] = st
                st = emit_bn_stats.st[k]
                nc.vector.bn_stats(out=st[:, j, :],
                                   in_=xs[k][:, j * 512:(j + 1) * 512])
            emit_bn_stats.st = [None] * CK

            def emit_act_stats(xs, hctx, k):
                """Batch-0 startup only: channel sums/sumsq on ACT (h tile
                used as scratch), freeing DVE of half the bn_stats."""
                hs = hctx[0]
                sq = small.tile([128, 1], F32, tag=f"sq{k}", name=f"sq{k}")
                sm = small.tile([128, 1], F32, tag=f"sm{k}", name=f"sm{k}")
                nc.scalar.activation(out=hs[k], in_=xs[k], func=AF.Square,
                                     accum_out=sq)
                nc.scalar.activation(out=hs[k], in_=xs[k], func=AF.Copy,
                                     accum_out=sm)
                emit_act_stats.acc[k] = (sm, sq)
            emit_act_stats.acc = [None] * CK

            def stats_finish(b, act_chunks=()):
                """bn_aggr + group reduce -> per-channel sc/nb coeffs."""
                rhs2s = []
                for k in range(CK):
                    r2 = small.tile([128, 2], F32, tag=f"r2{k}")
                    if k in act_chunks:
                        sm, sq = emit_act_stats.acc[k]
                        nc.vector.tensor_scalar_mul(
                            out=r2[:, 0:1], in0=sm, scalar1=float(1.0 / N))
                        nc.vector.tensor_scalar_mul(
                            out=r2[:, 1:2], in0=sq, scalar1=float(1.0 / N))
                        rhs2s.append(r2)
                        continue
                    mv = small.tile([128, 2], F32, tag=f"mv{k}")
                    nc.vector.bn_aggr(out=mv, in_=emit_bn_stats.st[k])
                    nc.gpsimd.tensor_copy(out=r2[:, 0:1], in_=mv[:, 0:1])
                    nc.vector.scalar_tensor_tensor(
                        out=r2[:, 1:2], in0=mv[:, 0:1],
                        scalar=mv[:, 0:1], in1=mv[:, 1:2],
                        op0=OP.mult, op1=OP.add)
                    rhs2s.append(r2)
                pg_t = ps_w.tile([128, 512], F32, tag="wk")
                pg = pg_t[0:G, 0:2]
                for k in range(CK):
                    nc.tensor.matmul(pg, indf[k], rhs2s[k],
                                     start=(k == 0), stop=(k == CK - 1))
                sg = small.tile([G, 2], F32, tag="sg")
                nc.vector.tensor_copy(out=sg, in_=pg)
                t2 = small.tile([G, 1], F32, tag="t2")
                nc.vector.tensor_mul(out=t2, in0=sg[:, 0:1], in1=sg[:, 0:1])
                vs = small.tile([G, 1], F32, tag="vs")
                nc.vector.tensor_sub(out=vs, in0=sg[:, 1:2], in1=t2)
                lnv = small.tile([G, 1], F32, tag="lnv")
                nc.scalar.activation(out=lnv, in_=vs, func=AF.Ln,
                                     bias=eps_t[0:G, :], scale=1.0)
                rstd = small.tile([G, 1], F32, tag="rstd")
                nc.scalar.activation(out=rstd, in_=lnv, func=AF.Exp, scale=-0.5)
                bcr = small.tile([G, 2], F32, tag="bcr")
                nc.gpsimd.tensor_copy(out=bcr[:, 0:1], in_=sg[:, 0:1])
                nc.gpsimd.tensor_copy(out=bcr[:, 1:2], in_=rstd)
                scs = []
                nbs = []
                for k in range(CK):
                    pbc_t = ps_w.tile([128, 512], F32, tag="wk")
                    pbc = pbc_t[:, 0:2]
                    nc.tensor.matmul(pbc, indb[:, k * 128:(k + 1) * 128], bcr,
                                     start=True, stop=True)
                    sc = small.tile([128, 1], F32, tag=f"sc{k}")
                    nc.vector.tensor_mul(out=sc, in0=pbc[:, 1:2], in1=gam[k])
                    t4 = small.tile([128, 1], F32, tag=f"t4{k}")
                    nc.vector.tensor_scalar_mul(out=t4, in0=pbc[:, 0:1],
                                                scalar1=sc)
                    nb = small.tile([128, 1], F32, tag=f"nb{k}")
                    nc.vector.tensor_sub(out=nb, in0=bet[k], in1=t4)
                    scs.append(sc)
                    nbs.append(nb)
                return scs, nbs

            def alloc_h(b):
                hs = [hpool.tile([128, N], BF, tag=f"h{k}", name=f"h{k}")
                      for k in range(CK)]
                h8 = [hpool.tile([128, 2, N], F8, tag=f"h8{p}", name=f"h8{p}")
                      for p in range(NP)]
                hps = [small.tile([128, 2], F32, tag=f"hp{k}", name=f"hp{k}")
                       for k in range(CK)]
                return hs, h8, hps

            def norm_act(xs, hctx, scs, nbs, k, half):
                """One [128,2048] normalize op on ACT (+accum partial)."""
                hs, h8, hps = hctx
                sl = slice(half * 2048, (half + 1) * 2048)
                nc.scalar.activation(
                    out=hs[k][:, sl], in_=xs[k][:, sl], func=AF.Identity,
                    bias=nbs[k], scale=scs[k],
                    accum_out=hps[k][:, half:half + 1])

            def h8_op(xs, hctx, scs, nbs, k, half):
                """Matching fp8 normalize on Pool (independent of norm_act)."""
                hs, h8, hps = hctx
                sl = slice(half * 2048, (half + 1) * 2048)
                nc.gpsimd.tensor_scalar(
                    out=h8[k // 2][:, k % 2, sl], in0=xs[k][:, sl],
                    scalar1=scs[k], scalar2=nbs[k], op0=OP.mult, op1=OP.add)

            def hsum_finish(hctx):
                hs, h8, hps = hctx
                hsums = []
                for k in range(CK):
                    t = small.tile([128, 1], BF, tag=f"hsum{k}")
                    nc.vector.tensor_add(out=t, in0=hps[k][:, 0:1],
                                         in1=hps[k][:, 1:2])
                    hsums.append(t)
                return hsums

            def transpose_chunk(b, hctx, s):
                hs, h8, hps = hctx
                # 4 transposes of h[:, s*128:(s+1)*128] into one bf16 psum tile
                pht = ps_tr.tile([128, 512], BF, tag="tr")
                for k in range(CK):
                    nc.tensor.transpose(
                        pht[:, k * 128:(k + 1) * 128],
                        hs[k][:, s * 128:(s + 1) * 128], ident)
                hT = mid.tile([128, 512], BF, tag="hT", bufs=4)
                nc.vector.tensor_copy(out=hT, in_=pht)
                return hT

            def gram_chunk(b, Gs, hT, s):
                # G[ck] += hT[:, ck].T @ hT -- upper-block-triangular only
                # (G symmetric: lower blocks are filled by transposes later)
                for k in range(CK):
                    nc.tensor.matmul(Gs[k][:, k * 128:],
                                     hT[:, k * 128:(k + 1) * 128],
                                     hT[:, k * 128:],
                                     start=(s == 0), stop=(s == SP - 1))

            def vproj_t(b, hctx, t):
                hs, h8, hps = hctx
                vts = []
                for oc in range(CK):
                    pv = ps_w.tile([128, 512], F32,
                                   tag=("av" if oc % 2 else "wk"), name="pv")
                    for p in range(NP):
                        nc.tensor.matmul(
                            pv, wv8[p][:, :, oc * 128:(oc + 1) * 128],
                            h8[p][:, :, t * 512:(t + 1) * 512],
                            start=(p == 0), stop=(p == NP - 1), perf_mode=DR)
                    vt = vpool.tile([128, 512], BF, tag=f"v{oc}_{t}")
                    nc.scalar.mul(out=vt, in_=pv, mul=IWSC)
                    vts.append(vt)
                return vts

            def gram_finish(b, Gs, hsums):
                # qks row = hsum^T @ wqkT  -> [1, 2C]  (independent of G)
                pq_t = ps_w.tile([128, 512], F32, tag="av")
                qks_sb = gpool.tile([1, 2 * C], BF, tag="qks")
                for half in range(2):
                    pq = pq_t[0:1, :]
                    for k in range(CK):
                        nc.tensor.matmul(
                            pq, hsums[k],
                            wqk[k][:, half * 512:(half + 1) * 512],
                            start=(k == 0), stop=(k == CK - 1))
                    nc.vector.tensor_copy(
                        out=qks_sb[:, half * 512:(half + 1) * 512], in_=pq)
                # evac computed (upper) G blocks, then mirror the lower
                # blocks via PE transposes of the uppers
                G_sb = []
                for k in range(CK):
                    t = gpool.tile([128, 512], BF, tag=f"G{k}")
                    nc.scalar.copy(out=t[:, k * 128:], in_=Gs[k][:, k * 128:])
                    G_sb.append(t)
                # mirror: G_sb[m][:, a*128:(a+1)*128] = G_sb[a][:, m-block].T
                pmir1 = ps_tr.tile([128, 512], BF, tag="tr", name="pmir1")
                mir1 = [(1, 0), (2, 0), (2, 1), (3, 0)]
                for i, (m, a) in enumerate(mir1):
                    nc.tensor.transpose(
                        pmir1[:, i * 128:(i + 1) * 128],
                        G_sb[a][:, m * 128:(m + 1) * 128], ident)
                pmir2 = ps_tr.tile([128, 512], BF, tag="tr", name="pmir2")
                for i, (m, a) in enumerate([(3, 1), (3, 2)]):
                    nc.tensor.transpose(
                        pmir2[:, i * 128:(i + 1) * 128],
                        G_sb[a][:, m * 128:(m + 1) * 128], ident)
                nc.vector.tensor_copy(out=G_sb[1][:, 0:128],
                                      in_=pmir1[:, 0:128])
                nc.vector.tensor_copy(out=G_sb[2][:, 0:256],
                                      in_=pmir1[:, 128:384])
                nc.scalar.copy(out=G_sb[3][:, 0:128], in_=pmir1[:, 384:512])
                nc.scalar.copy(out=G_sb[3][:, 128:384], in_=pmir2[:, 0:256])
                # ks2 = ks + N*bk
                ks2 = gpool.tile([1, C], BF, tag="ks2")
                nc.vector.scalar_tensor_tensor(
                    out=ks2, in0=bqkr[:, C:2 * C], scalar=float(N),
                    in1=qks_sb[:, C:2 * C], op0=OP.mult, op1=OP.add)
                # T = G @ Wk^T  (G symmetric: lhsT = G_sb[a][:, m-chunk])
                T_sb = []
                for m in range(CK):
                    pT = ps_w.tile([128, 512], F32, tag="wk")
                    for a in range(CK):
                        nc.tensor.matmul(
                            pT, G_sb[a][:, m * 128:(m + 1) * 128],
                            wqk[a][:, C:2 * C],
                            start=(a == 0), stop=(a == CK - 1))
                    t = gpool.tile([128, 512], BF, tag=f"T{m}")
                    nc.scalar.copy(out=t, in_=pT)
                    T_sb.append(t)
                # scores: per chunk ck, heads 2ck (even rows) / 2ck+1 (odd)
                # shares the "av" bank: qks (before) and cv (after) don't
                # overlap its lifetime
                SC = ps_w.tile([128, 256], F32, tag="av")
                for ck in range(CK):
                    for par in range(2):
                        hh = 2 * ck + par
                        hsl = slice(hh * 64, (hh + 1) * 64)
                        out_ap = SC[par * 64:(par + 1) * 64,
                                    ck * 64:(ck + 1) * 64]
                        tp = (0, par * 64)
                        nc.tensor.matmul(
                            out_ap, bqkr[:, hsl], ks2[:, hsl],
                            start=True, stop=False, tile_position=tp,
                            skip_group_check=True)
                        nc.tensor.matmul(
                            out_ap, qks_sb[:, hsl], bqkr[:, C + hh * 64:
                                                         C + (hh + 1) * 64],
                            start=False, stop=False, tile_position=tp,
                            skip_group_check=True)
                        for a in range(CK):
                            nc.tensor.matmul(
                                out_ap, wqk[a][:, hsl], T_sb[a][:, hsl],
                                start=False, stop=(a == CK - 1),
                                tile_position=tp, skip_group_check=True)
                return SC

            def softmax(b, SC):
                p_f = mid.tile([128, 256], F32, tag="pf", bufs=1)
                rs = mid.tile([128, CK], F32, tag="rs", bufs=1)
                for ck in range(CK):
                    for par in range(2):
                        rsl = slice(par * 64, (par + 1) * 64)
                        nc.scalar.activation(
                            out=p_f[rsl, ck * 64:(ck + 1) * 64],
                            in_=SC[rsl, ck * 64:(ck + 1) * 64],
                            func=AF.Exp, scale=scale,
                            accum_out=rs[rsl, ck:ck + 1])
                rv = mid.tile([128, CK], F32, tag="rv", bufs=1)
                nc.vector.reciprocal(out=rv, in_=rs)
                for ck in range(CK):
                    for par in range(2):
                        rsl = slice(par * 64, (par + 1) * 64)
                        nc.vector.tensor_scalar_mul(
                            out=att_bf[ck][rsl, par * 64:(par + 1) * 64],
                            in0=p_f[rsl, ck * 64:(ck + 1) * 64],
                            scalar1=rv[rsl, ck:ck + 1])

            def att_transpose(b):
                patt = ps_tr.tile([128, 512], BF, tag="tr")
                for ck in range(CK):
                    nc.tensor.transpose(
                        patt[:, ck * 128:(ck + 1) * 128], att_bf[ck], ident)
                attT = mid.tile([128, 512], BF, tag="attT", bufs=1)
                nc.vector.tensor_copy(out=attT, in_=patt)
                # cv = attT.T(!) applied to b_v: one matmul per chunk
                pcv_t = ps_w.tile([128, 512], F32, tag="av")
                for ck in range(CK):
                    nc.tensor.matmul(
                        pcv_t[:, ck:ck + 1],
                        attT[:, ck * 128:(ck + 1) * 128], bv_sb[ck],
                        start=True, stop=True, skip_group_check=True)
                cs4 = small.tile([128, CK], F32, tag="cs4")
                nc.vector.tensor_copy(out=cs4, in_=pcv_t[:, 0:CK])
                return attT, cs4

            def attv_t(b, attT, cs4, vsave, hv8, t, split_act=False):
                # att @ v for the 4 chunks of this t-block; psum rotates
                # through the (now free) G bank tags for a 4-deep pipeline
                for ck in range(CK):
                    pav = ps_g.tile([128, 512], F32, tag=f"G{ck}",
                                    name=f"pav{ck}")
                    nc.tensor.matmul(
                        pav, attT[:, ck * 128:(ck + 1) * 128],
                        vsave[t][ck], start=True, stop=True)
                    hv_ap = hv8[ck // 2][:, ck % 2, t * 512:(t + 1) * 512]
                    if split_act and ck % 2:
                        nc.scalar.activation(out=hv_ap, in_=pav,
                                             func=AF.Identity,
                                             bias=cs4[:, ck:ck + 1], scale=1.0)
                    else:
                        nc.vector.tensor_scalar_add(out=hv_ap, in0=pav,
                                                    scalar1=cs4[:, ck:ck + 1])

            def outproj_t(b, hv8, t):
                for oc in range(CK):
                    po = ps_w.tile([128, 512], F32,
                                   tag=("av" if oc % 2 else "wk"),
                                   name=f"po{oc}")
                    for p in range(NP):
                        nc.tensor.matmul(
                            po, wo8[p][:, :, oc * 128:(oc + 1) * 128],
                            hv8[p][:, :, t * 512:(t + 1) * 512],
                            start=(p == 0), stop=(p == NP - 1), perf_mode=DR)
                    xr = mid.tile([128, 512], F32, tag="xr", bufs=2)
                    nc.sync.dma_start(
                        out=xr,
                        in_=xf_d.ap()[b, oc * 128:(oc + 1) * 128,
                                      t * 512:(t + 1) * 512])
                    fin = mid.tile([128, 512], F32, tag="fin", bufs=2)
                    if oc % 2 == 0:
                        # P1: ACT (po/32 + bo) -> Pool (+x) -> gpsimd store
                        ot = mid.tile([128, 512], BF, tag="ot", bufs=2)
                        nc.scalar.activation(out=ot, in_=po, func=AF.Identity,
                                             bias=bo_sb[oc], scale=IWSC)
                        nc.gpsimd.tensor_add(out=fin, in0=xr, in1=ot)
                        dma_eng = nc.gpsimd
                    else:
                        # P2: DVE (po/32 + x) -> ACT (+bo) -> sync store
                        t1 = mid.tile([128, 512], F32, tag="t1", bufs=2)
                        nc.vector.scalar_tensor_tensor(
                            out=t1, in0=po, scalar=IWSC, in1=xr,
                            op0=OP.mult, op1=OP.add)
                        nc.scalar.activation(out=fin, in_=t1,
                                             func=AF.Identity,
                                             bias=bo_sb[oc], scale=1.0)
                        dma_eng = nc.sync
                    dma_eng.dma_start(
                        out=out_d.ap()[b, oc * 128:(oc + 1) * 128,
                                       t * 512:(t + 1) * 512],
                        in_=fin)

            # ---------------- pipelined emission ----------------
            # batch 0 prologue
            xs = load_x(0)
            hctx = alloc_h(0)
            for k in range(2):
                for j in range(SUB):
                    emit_bn_stats(xs, k, j)
            for k in range(2, CK):
                emit_act_stats(xs, hctx, k)
            scs, nbs = stats_finish(0, act_chunks=(2, 3))
            for k in range(CK):
                h8_op(xs, hctx, scs, nbs, k, 0)
            for half in range(2):
                for k in range(CK):
                    norm_act(xs, hctx, scs, nbs, k, half)
            for k in range(CK):
                h8_op(xs, hctx, scs, nbs, k, 1)
            hsums = hsum_finish(hctx)

            prev_hv8 = None  # previous batch's hv8 (out-proj deferred)
            for b in range(B):
                last = (b == B - 1)
                Gs = [ps_g.tile([128, 512], F32, tag=f"G{k}", name=f"G{k}")
                      for k in range(CK)]
                vsave = [None] * NT
                hv8 = [hpool.tile([128, 2, N], F8, tag=f"hv8{p}",
                                  name=f"hv8{p}")
                       for p in range(NP)]
                nxt_stats_ops = []
                if not last:
                    nxt_stats_ops = [(k, j) for k in range(CK)
                                     for j in range(SUB)]
                GLAG = 2  # transposes run 2 chunks ahead of G matmuls
                hT_q = []
                for s in range(SP + GLAG):
                    if not last and s == 0:
                        xs_n = load_x(b + 1)
                    if s < SP:
                        hT_q.append(transpose_chunk(b, hctx, s))
                    if s >= GLAG:
                        gram_chunk(b, Gs, hT_q[s - GLAG], s - GLAG)
                    if s % 4 == 3:
                        t = s // 4
                        if not last:
                            # last batch defers vproj past gram_finish so
                            # the softmax chain starts as early as possible
                            vsave[t] = vproj_t(b, hctx, t)
                        if prev_hv8 is not None and t < NT:
                            outproj_t(b - 1, prev_hv8, t)
                    if not last and s >= 8:
                        # spread next batch's bn_stats: 1/s then 2/s
                        nops = 1 if s < 24 else 2
                        for _ in range(nops):
                            if nxt_stats_ops:
                                k, j = nxt_stats_ops.pop(0)
                                emit_bn_stats(xs_n, k, j)
                prev_hv8 = None
                SCp = gram_finish(b, Gs, hsums)
                softmax(b, SCp)
                if last:
                    for t in range(NT):
                        vsave[t] = vproj_t(b, hctx, t)
                else:
                    scs, nbs = stats_finish(b + 1)
                    hctx_n = alloc_h(b + 1)
                    # h8 half-0 first so the next spatial loop's vproj t=0
                    # has its input early
                    for k in range(CK):
                        h8_op(xs_n, hctx_n, scs, nbs, k, 0)
                    for half in range(2):
                        for k in range(CK):
                            norm_act(xs_n, hctx_n, scs, nbs, k, half)
                    for k in range(CK):
                        h8_op(xs_n, hctx_n, scs, nbs, k, 1)
                attT, cs4 = att_transpose(b)
                for t in range(NT):
                    attv_t(b, attT, cs4, vsave, hv8, t, split_act=last)
                if last:
                    for t in range(NT):
                        outproj_t(b, hv8, t)
                else:
                    prev_hv8 = hv8
                    hctx = hctx_n
                    hsums = hsum_finish(hctx)

    nc.compile()
    return nc


def make_indicators():
    ch = np.arange(C)
    grp = ch // (C // G)
    indf = np.zeros((C, G), np.float32)
    indf[ch, grp] = 1.0 / (C // G)
    indb = np.zeros((G, C), np.float32)
    indb[grp, ch] = 1.0
    return indf, indb


def prep_weights(w_qkv, b_qkv, w_out, b_out, gamma, beta):
    """Host-side weight layouts. Returns dict of per-core input tensors
    (excluding x)."""
    bf = ml_dtypes.bfloat16
    f8 = ml_dtypes.float8_e4m3
    w_qkv = np.asarray(w_qkv, np.float32)
    wqkT = np.ascontiguousarray(w_qkv[:2 * C].T).astype(bf)

    def pack_dr(wT):
        # wT [C, C] (contraction-major) -> [NP, 128, 2, C] fp8 scaled
        a = (np.asarray(wT, np.float32) * WSC).reshape(NP, 2, 128, C)
        return np.ascontiguousarray(a.transpose(0, 2, 1, 3)).astype(f8)

    wv8 = pack_dr(w_qkv[2 * C:].T)
    wo8 = pack_dr(np.asarray(w_out, np.float32).T)
    b_qkv = np.asarray(b_qkv, np.float32)
    indf, indb = make_indicators()
    return {
        "wqkT": wqkT, "wv8": wv8, "wo8": wo8,
        "bqkr": np.ascontiguousarray(b_qkv[:2 * C].reshape(1, -1)).astype(bf),
        "bv": np.ascontiguousarray(b_qkv[2 * C:].reshape(-1, 1)).astype(bf),
        "bo": np.ascontiguousarray(np.asarray(b_out, np.float32).reshape(-1, 1)),
        "gamma": np.ascontiguousarray(np.asarray(gamma, np.float32).reshape(-1, 1)),
        "beta": np.ascontiguousarray(np.asarray(beta, np.float32).reshape(-1, 1)),
        "indf": indf, "indb": indb,
    }


_PROGRAM = None


def _get_program():
    global _PROGRAM
    if _PROGRAM is None:
        _PROGRAM = build_program()
    return _PROGRAM


def kernel(x, gamma, beta, w_qkv, b_qkv, w_out, b_out):
    x = np.asarray(x)
    B, C_, H, W = x.shape
    N = H * W
    assert C_ == C and B == 16 and N == 4096
    nc = _get_program()

    bf = ml_dtypes.bfloat16
    wd = prep_weights(w_qkv, b_qkv, w_out, b_out, gamma, beta)
    xr = np.ascontiguousarray(x.reshape(B, C, N).astype(np.float32))
    xb = xr.astype(bf)

    bpc = B // N_CORES
    in_maps = []
    for c in range(N_CORES):
        m = {"xbf": xb[c * bpc:(c + 1) * bpc],
             "xf": xr[c * bpc:(c + 1) * bpc]}
        m.update(wd)
        in_maps.append(m)
    res = run_bass_kernel_spmd(nc, in_maps, core_ids=list(range(N_CORES)))
    out = np.concatenate([res.results[c]["out"] for c in range(N_CORES)],
                         axis=0)
    return out.reshape(B, C_, H, W).astype(np.float32)
